# revision 3
# baseline (speedup 1.0000x reference)
"""AM sign-quantize hamming kernel, v3.

logit[b, c] = (D + sum_d sign(q[b,d]) * sign(am[c,d])) / 2

Per-core structure (8-way batch-parallel q, kt-sharded am all-gather):
  - am: each core gets a [1024 padded classes, 1280 d] column shard
    (host-sliced), fp8 cast-load pair-interleaved, DoubleRow identity
    transpose on PE.  The psum evac is ONE-SIDED: {0, m} via
    (x>0)*m on DVE, m = 0.25/|q| per kt-residue, so products are
    uniformly +-0.25 / 0 and the a<=0 half is reconstructed from the
    per-row q sign-sum S:  logit = 4*P + (D/2 - S/2).
  - kt-sharding makes the all-gather output kt-major: matmul chains
    start right after the first regathered slice; two AllGathers split
    by class half (cols 0:512 / 512:1024) pipeline on the collective
    cores.
  - q: 4 pair-tiles of 256 rows, fp8 cast-load pair-interleaved,
    DoubleRow identity transpose; sign in the psum evac, spread over
    Act (Sign -> +-1) and DVE ((x>0)-0.5 -> +-0.5) by kt-pair residue
    mod 5 (SPMD-uniform; GPSIMD cannot read PSUM on real hw).
  - S is accumulated by N=1 matmuls (weights 1 or 2 by residue) into
    psum column 500 of each half-0 chain bank; t = D/2 - S/2 parks in
    SBUF and returns as the per-partition bias of the output evacs
    (Act Relu(4P+t), exact since logits >= 0, or DVE mult/add).
  - main matmul: fp8 DoubleRow, per 128-row block 2 chains of N=500,
    accumulating 40 kt-pairs in regather-slice order; int16 out.
"""

import os
import sys

if "/opt/trn_rl_repo" not in sys.path:
    sys.path.insert(0, "/opt/trn_rl_repo")

import numpy as np

from concourse import bacc, bass, masks, mybir
from concourse.bass_utils import run_bass_kernel_spmd
from concourse.tile import TileContext

B, D, C = 8192, 10240, 1000
NCORES = 8
BS = B // NCORES  # 1024 batch rows per core
CPAD = 1024  # padded classes
KT = D // 128  # 80 k-tiles
KSH = KT // NCORES  # 10 k-tiles per core shard
DSH = KSH * 128  # 1280 d-columns per shard
KP = KT // 2  # 40 kt-pairs

F32 = mybir.dt.float32
FP8 = mybir.dt.float8e4
I16 = mybir.dt.int16
BF16 = mybir.dt.bfloat16

gt = mybir.AluOpType.is_gt
add = mybir.AluOpType.add
sub = mybir.AluOpType.subtract
mult = mybir.AluOpType.mult
band = mybir.AluOpType.bitwise_and
bor = mybir.AluOpType.bitwise_or
copyf = mybir.ActivationFunctionType.Copy
signf = mybir.ActivationFunctionType.Sign
reluf = mybir.ActivationFunctionType.Relu
DR = mybir.MatmulPerfMode.DoubleRow

QT_BUFS = int(os.environ.get("QT_BUFS", "4"))
QLD_BUFS = int(os.environ.get("QLD_BUFS", "6"))
PS_T_BUFS = int(os.environ.get("PS_T_BUFS", "4"))
PS_MM_BUFS = int(os.environ.get("PS_MM_BUFS", "4"))
# per kt-pair residue mod 5 (SPMD-uniform: each core owns 5 kt-pairs):
# (q_engine, q_magnitude, am_engine, am_magnitude); products all +-0.25.
# 'a' = Act Sign (+-1 only), 'v' = DVE, 'p' = Pool (bitwise sign-extract).
# per kt-pair residue mod 5 (SPMD-uniform: each core owns 5 kt-pairs):
# (q_engine, S_weight, am_magnitude).  q on Act -> Sign (+-1), S-weight 1;
# q on DVE -> (x>0)-0.5 (+-0.5), S-weight 2.  am is evacuated ONE-SIDED
# {0, m} with m = 0.25/|q| so every matmul product is +-0.25 or 0, and the
# missing a<=0 half is reconstructed from the per-row q sign-sum S:
#   logit = (D + 2*sum_{a>0} sq - S) / 2 = 4*P + (D/2 - S/2)
LTAB = [
    ("a", 1.0, 0.25),
    ("v", 2.0, 0.5),
    ("a", 1.0, 0.25),
    ("v", 2.0, 0.5),
    ("a", 1.0, 0.25),
]
N0 = N1 = 500
SCOL = 500

DCH = 2560  # q load chunk along D
NCH = D // DCH  # 4
KPCH = DCH // 256  # 10 kt-pairs per chunk





def build_nc() -> bass.Bass:
    nc = bacc.Bacc(None, target_bir_lowering=False, num_devices=NCORES)
    q_ext = nc.declare_dram_parameter("query", [BS, D], F32, isOutput=False)
    am_ext = nc.declare_dram_parameter("am_weight", [CPAD, DSH], F32, isOutput=False)
    out_ext = nc.declare_dram_parameter("out", [BS, C], I16, isOutput=True)

    with TileContext(nc) as tc:
        with (
            tc.tile_pool(name="const", bufs=1) as constp,
            tc.tile_pool(name="sat", bufs=1) as satp,
            tc.tile_pool(name="dram", bufs=1, space="DRAM") as dramp,
            tc.tile_pool(name="ld", bufs=QLD_BUFS) as ldp,
            tc.tile_pool(name="qt", bufs=1) as qtp,
            tc.tile_pool(name="outp", bufs=2) as outp,
            tc.tile_pool(name="ps_t", bufs=PS_T_BUFS, space="PSUM") as ps_t,
            tc.tile_pool(name="ps_mm", bufs=PS_MM_BUFS, space="PSUM") as ps_mm,
        ):
            ident = constp.tile([128, 128], BF16)
            masks.make_identity(nc, ident[:])
            # I256[p, ko, n] = 1 iff n == 2p+ko : transposes 256
            # pair-interleaved rows per DoubleRow pass
            i256 = constp.tile([128, 2, 256], FP8)
            nc.vector.memset(i256[:], 0.0)
            for ko in range(2):
                nc.vector.tensor_scalar(
                    i256[:, ko, ko::2], ident[:], 0.0, None, add
                )

            wA = constp.tile([128, 2, 1], FP8)
            nc.vector.memset(wA[:], 1.0)
            wV = constp.tile([128, 2, 1], FP8)
            nc.vector.memset(wV[:], 2.0)
            saT = satp.tile([128, KT, CPAD], FP8)  # gathered signs, kt-major
            saTs = satp.tile([128, KSH, CPAD], FP8)  # this core's shard
            b_in0 = dramp.tile([128, KSH, 512], FP8)
            b_in1 = dramp.tile([128, KSH, 512], FP8)
            b_out0 = dramp.tile([NCORES, 128, KSH, 512], FP8, addr_space="Shared")
            b_out1 = dramp.tile([NCORES, 128, KSH, 512], FP8, addr_space="Shared")
            b_in = [b_in0, b_in1]
            b_out = [b_out0, b_out1]

            # ---- phase A: am shard -> sign -> transpose -> all-gather -----
            def am_half(h):
                for cb in range(2 * h, 2 * h + 2):
                    a8 = ldp.tile([128, 2, DSH], FP8, tag="ld")
                    asrc = am_ext[cb * 256 : (cb + 1) * 256, :].rearrange(
                        "(p two) d -> p two d", two=2
                    )
                    nc.gpsimd.dma_start(out=a8[:], in_=asrc)
                    for g in range(KSH // 2):  # local kt-pair
                        pt = ps_t.tile([128, 2, 256], F32, tag="ps_t")
                        for j in range(2):
                            kl = g * 2 + j
                            nc.tensor.matmul(
                                pt[:, j, :],
                                a8[:, :, kl * 128 : (kl + 1) * 128],
                                i256[:],
                                start=(j == 0),
                                stop=(j == 1),
                                skip_group_check=True,
                                perf_mode=DR,
                            )
                        mag = LTAB[g][2]
                        dst = saTs[:, 2 * g : 2 * g + 2, cb * 256 : (cb + 1) * 256]
                        nc.vector.tensor_scalar(dst, pt[:], 0.0, mag, gt, mult)

            def q_load(mt, ch):
                b0 = mt * 256
                src = q_ext[
                    b0 : b0 + 256, ch * DCH : (ch + 1) * DCH
                ].rearrange("(p two) d -> p two d", two=2)
                qf = ldp.tile([128, 2, DCH], FP8, tag="ld")
                nc.gpsimd.dma_start(out=qf[:], in_=src)
                return qf

            def q_pair(qf, ch, g, qT):
                kp = ch * KPCH + g
                pt = ps_t.tile([128, 2, 256], F32, tag="ps_t")
                for j in range(2):
                    nc.tensor.matmul(
                        pt[:, j, :],
                        qf[:, :, (2 * g + j) * 128 : (2 * g + j + 1) * 128],
                        i256[:],
                        start=(j == 0),
                        stop=(j == 1),
                        skip_group_check=True,
                        perf_mode=DR,
                    )
                dst = qT[:, 2 * kp : 2 * kp + 2, :]
                if LTAB[kp % 5][0] == "a":
                    nc.scalar.activation(dst, pt[:], signf)
                else:
                    nc.vector.tensor_scalar(dst, pt[:], 0.0, 0.5, gt, sub)

            def q_chunk(mt, ch, qT):
                qf = q_load(mt, ch)
                for g in range(KPCH):
                    q_pair(qf, ch, g, qT)

            def q_tile_work(mt, qT, chlo=0, chhi=NCH):
                """Yield thunks: transposes for chunks [chlo, chhi) of one
                pair-tile (loads already issued upfront), for interleaving
                into a matmul segment."""

                def pair(ch, g):
                    def f():
                        q_pair(qfs[(mt, ch)], ch, g, qT)
                    return f

                for ch in range(chlo, chhi):
                    for g in range(KPCH):
                        yield pair(ch, g)

            with tc.high_priority():
                am_half(0)
                am_half(1)
                for h in range(2):
                    nc.sync.dma_start(
                        out=b_in[h][:], in_=saTs[:, :, h * 512 : h * 512 + 512]
                    )
                    nc.gpsimd.collective_compute(
                        "AllGather",
                        mybir.AluOpType.bypass,
                        replica_groups=[list(range(NCORES))],
                        ins=[b_in[h][:].opt()],
                        outs=[b_out[h][:].opt()],
                    )
                for h in range(2):
                    for s in range(NCORES):
                        nc.sync.dma_start(
                            out=saT[
                                :, s * KSH : (s + 1) * KSH, h * 512 : h * 512 + 512
                            ],
                            in_=b_out[h][s],
                        )

            # ---- phase B: all q loads issued upfront (their SWDGE preps
            # must not queue behind Pool evacs), then transposes ------------
            qTs = []
            for mt in range(4):
                qTs.append(
                    qtp.tile([128, KT, 256], FP8, name=f"qT{mt}", tag=f"qt{mt}")
                )
            qfs = {}
            for mt in range(4):
                for ch in range(NCH):
                    qfs[(mt, ch)] = q_load(mt, ch)
            for mt in range(2):
                for ch in range(NCH):
                    for g in range(KPCH):
                        q_pair(qfs[(mt, ch)], ch, g, qTs[mt])
            for ch in range(2):
                for g in range(KPCH):
                    q_pair(qfs[(2, ch)], ch, g, qTs[2])

            tvec = satp.tile([128, 8], F32)  # per row-block: D/2 - S/2

            # ---- phase C: main matmuls ------------------------------------
            # chain(mt, mb, half): accumulate 40 kt-pairs in slice order,
            # with the next pair-tile's loads/transposes interleaved so PE
            # has matmul work while the transpose evacs drain.
            evac_rr = [0]

            def evac_store(pm, mt, mb, half, c0, nn, split=False):
                rb = mt * 2 + mb
                t_ap = tvec[:, rb : rb + 1]
                pieces = [(0, nn // 2), (nn // 2, nn)] if split else [(0, nn)]
                for lo, hi in pieces:
                    ot = outp.tile([128, hi - lo], I16, tag="ot")
                    # logit = 4*P + (D/2 - S/2)
                    rr = evac_rr[0] = (evac_rr[0] + 1) % 2
                    if rr == 0:
                        # logits are >= 0, so Relu(4P + t) is exact (Copy
                        # rejects AP bias)
                        nc.scalar.activation(
                            ot[:], pm[:, lo:hi], reluf, bias=t_ap, scale=4.0
                        )
                    else:
                        nc.vector.tensor_scalar(
                            ot[:], pm[:, lo:hi], 4.0, t_ap, mult, add
                        )
                    rows = out_ext[
                        mt * 256 + mb * 128 : mt * 256 + (mb + 1) * 128,
                        c0 + lo : c0 + hi,
                    ]
                    eng = nc.sync if (mt + mb + half + lo) % 2 == 0 else nc.scalar
                    eng.dma_start(out=rows, in_=ot[:])

            def chain_kps(pm, qT, mb, c0, nn, kplo, kphi, with_s=False):
                for kp in range(kplo, kphi):
                    lhsT = qT[:, 2 * kp : 2 * kp + 2, mb * 128 : (mb + 1) * 128]
                    nc.tensor.matmul(
                        pm[:, 0:nn],
                        lhsT,
                        saT[:, 2 * kp : 2 * kp + 2, c0 : c0 + nn],
                        start=(kp == 0),
                        stop=(kp == KP - 1 and not with_s),
                        skip_group_check=True,
                        perf_mode=DR,
                    )
                    if with_s:
                        # S accumulation: S[b] = sum_d w * qT[d, b]
                        w = wA if LTAB[kp % 5][0] == "a" else wV
                        nc.tensor.matmul(
                            pm[:, SCOL : SCOL + 1],
                            lhsT,
                            w[:],
                            start=False,
                            stop=(kp == KP - 1),
                            skip_group_check=True,
                            perf_mode=DR,
                        )

            def s_to_t(pm, mt, mb):
                rb = mt * 2 + mb
                # t = D/2 - S/2
                nc.vector.tensor_scalar(
                    tvec[:, rb : rb + 1], pm[:, SCOL : SCOL + 1],
                    -0.5, float(D) / 2, mult, add,
                )

            def chain_mms_slice_major(pairs, half, extra=()):
                # all chains advance one regather-slice at a time: right
                # after the collective, when slices arrive incrementally
                extra = list(extra)
                c0, nn = (0, N0) if half == 0 else (N0, N1)
                pms = {}
                for mt, mb in pairs:
                    pms[(mt, mb)] = ps_mm.tile(
                        [128, 512], F32, name=f"pm{mt}{mb}", tag="ps_mm"
                    )
                for s in range(NCORES):
                    for mt, mb in pairs:
                        gh = KSH // 2
                        chain_kps(
                            pms[(mt, mb)], qTs[mt], mb, c0, nn,
                            s * gh, (s + 1) * gh, with_s=(half == 0),
                        )
                    # a few interleaved transpose/load work items per slice
                    for _ in range(int(os.environ.get("THUNKS", "6"))):
                        if extra:
                            extra.pop(0)()
                for mt, mb in pairs:
                    if half == 0:
                        s_to_t(pms[(mt, mb)], mt, mb)
                    evac_store(pms[(mt, mb)], mt, mb, half, c0, nn)
                for f in extra:
                    f()

            def chain_mms_chain_major(pairs, half):
                # one chain at a time: staggers the final evacs/stores so
                # the kernel does not end with a burst
                c0, nn = (0, N0) if half == 0 else (N0, N1)
                for i, (mt, mb) in enumerate(pairs):
                    pm = ps_mm.tile([128, 512], F32, name=f"pm{mt}{mb}", tag="ps_mm")
                    chain_kps(pm, qTs[mt], mb, c0, nn, 0, KP, with_s=(half == 0))
                    if half == 0:
                        s_to_t(pm, mt, mb)
                    evac_store(pm, mt, mb, half, c0, nn)

            pairs01 = [(mt, mb) for mt in range(2) for mb in range(2)]
            pairs23 = [(mt, mb) for mt in range(2, 4) for mb in range(2)]

            import itertools

            chain_mms_slice_major(
                pairs01, 0,
                itertools.chain(
                    q_tile_work(2, qTs[2], 2, NCH), q_tile_work(3, qTs[3], 0, 2)
                ),
            )
            chain_mms_slice_major(pairs01, 1, q_tile_work(3, qTs[3], 2, NCH))
            chain_mms_chain_major(pairs23, 0)
            chain_mms_chain_major(pairs23, 1)

    nc.compile()
    return nc


def make_in_maps(query: np.ndarray, am_weight: np.ndarray):
    query = np.ascontiguousarray(query, dtype=np.float32)
    am_weight = np.ascontiguousarray(am_weight, dtype=np.float32)
    assert query.shape == (B, D), query.shape
    assert am_weight.shape == (C, D), am_weight.shape
    am_pad = np.zeros((CPAD, D), dtype=np.float32)
    am_pad[:C] = am_weight
    return [
        {
            "query": query[i * BS : (i + 1) * BS],
            "am_weight": np.ascontiguousarray(am_pad[:, i * DSH : (i + 1) * DSH]),
        }
        for i in range(NCORES)
    ]


_NC = None


def kernel(query: np.ndarray, am_weight: np.ndarray) -> np.ndarray:
    global _NC
    if _NC is None:
        _NC = build_nc()
    in_maps = make_in_maps(query, am_weight)
    res = run_bass_kernel_spmd(_NC, in_maps, core_ids=list(range(NCORES)))
    return np.concatenate(
        [res.results[i]["out"].astype(np.float32) for i in range(NCORES)], axis=0
    )


if __name__ == "__main__":
    q = np.random.randn(B, D).astype(np.float32)
    a = np.random.randn(C, D).astype(np.float32)
    out = kernel(q, a)
    sq = np.where(q > 0, 1.0, -1.0).astype(np.float32)
    sa = np.where(a > 0, 1.0, -1.0).astype(np.float32)
    ref = (D + sq @ sa.T) * 0.5
    err = np.abs(out - ref).max()
    print("max abs err:", err)
